# revision 13
# baseline (speedup 1.0000x reference)
"""ConditionAwareAdaIN Trainium2 kernel (v5: flipped stage-1 + XBAR
transpose, fp16 I/O, engine-balanced elementwise).

Reference computation (B=16, C=256, L=1024, U=64, Q=64):
    nx    = InstanceNorm1d(x)                       # per-(b,c) stats over L
    A     = einsum('bu,cuq->bcq', u_i, W.reshape(2C,U,Q))
    style = einsum('bcq,bql->bcl', A, e_qid)
    gamma, beta = split(style + V@t + bias, 2, axis=1)
    out   = (1 + gamma) * nx + beta

Sharding: 2-way over batch x 4-way over channels -> 8 cores, each owning
8 samples x 64 channels (its slice of gamma AND beta rows of W/V/bias).

Key points:
  - all device I/O tensors are fp16 -> DMA bytes halve vs fp32; PSUM and
    stats stay fp32.  End-to-end rel err ~1e-3 vs the 2e-2 gate.
  - stage 1 is flipped: 64 matmuls out[c2(128), b(8)] (lhsT = W_q slice,
    rhs = u_i^T), all into ONE psum bank -> a single [128, 512]
    evacuation instead of 8x1024 columns of [8, *] copies.
  - the A redistribute is a DMA-engine XBAR transpose (SBUF->SBUF,
    one per sample pair) -- no DRAM bounce -- followed by two cheap
    partition-preserving copies per pair into the zeroed block-diagonal
    lhsT tile.
  - per-pair elementwise: ACT evacuates P (rstd folded) and Q to fp16,
    Pool does (x-mean)*msb, DVE does stats + the final fp16 2x add; the
    last pair runs per-half straight through DVE to shorten the tail.
"""

import json

import numpy as np

for _p in ("/opt/trn_rl_repo", "/root/.axon_site/_ro/trn_rl_repo"):
    import sys as _sys

    if _p not in _sys.path:
        _sys.path.append(_p)

import concourse.bass as bass
import concourse.mybir as mybir
from concourse.tile import TileContext
from concourse.bass_utils import run_bass_kernel_spmd


def _split_sync_waits(raw: bytes, keep: int = 1) -> bytes:
    """Walrus in this env accepts at most one sync wait per TPB instruction.

    Tile packs several waits into sync_info.on_wait; re-emit the excess as
    standalone single-wait EventSemaphore instructions (what wait_ge emits)
    immediately before the instruction, in the same engine stream.
    """
    bir = json.loads(raw)
    n = 0
    for fn in bir["functions"]:
        for blk in fn["blocks"]:
            out = []
            for ins in blk["instructions"]:
                si = ins.get("sync_info")
                ws = si.get("on_wait") if si else None
                if ws and len(ws) > keep:
                    for w in ws[: len(ws) - keep]:
                        n += 1
                        out.append(
                            {
                                "debug": ins.get("debug", 0),
                                "engine": ins["engine"],
                                "ins": [],
                                "outs": [],
                                "name": f"evw-{n}",
                                "opcode": "EventSemaphore",
                                "sync_info": {"on_update": [], "on_wait": [w]},
                            }
                        )
                    si["on_wait"] = ws[len(ws) - keep :]
                out.append(ins)
            blk["instructions"] = out
    return json.dumps(bir).encode()


class _Bass(bass.Bass):
    def to_json_bytes(self) -> bytes:
        return _split_sync_waits(super().to_json_bytes())


B, C, L = 16, 256, 1024
U, Q = 64, 64
EPS = 1e-5
N_CORES = 8
BG, CG = 2, 4          # batch groups x channel groups
BPC = B // BG          # samples per core = 8
CPC = C // CG          # channels per core = 64
NPAIR = BPC // 2       # sample pairs per core = 4

FP32 = mybir.dt.float32
F16 = mybir.dt.float16

_CACHE = {}


def _build_nc(detect_races: bool = True):
    nc = _Bass(detect_race_conditions=detect_races)

    # xe: per pair, rows = (2 samples x 64 ch/q), cols = [x (1024) | e (1024)]
    xe_in = nc.dram_tensor("xe_s", [NPAIR, 128, 2 * L], F16, kind="ExternalInput")
    # wtq: rows u(64), cols q*128 + c2, value W2[c2, u, q]
    wt_in = nc.dram_tensor("wtq", [64, 8192], F16, kind="ExternalInput")
    # uiT: rows u(64), cols b(8)
    ui_in = nc.dram_tensor("uiT", [64, BPC], F16, kind="ExternalInput")
    # sm: [r2 (4 pairs x 1024) | l2 (256)] on 3 partitions
    sm_in = nc.dram_tensor("sm2", [3, NPAIR * L + 256], F16, kind="ExternalInput")
    out_d = nc.dram_tensor("out_s", [BPC, CPC, L], F16, kind="ExternalOutput")

    AF = mybir.ActivationFunctionType
    OP = mybir.AluOpType

    with TileContext(nc) as tc:
        with (
            tc.tile_pool(name="persist", bufs=1) as persist,
            tc.tile_pool(name="xe", bufs=4) as xe,
            tc.tile_pool(name="work", bufs=6) as work,
            tc.tile_pool(name="stat", bufs=8) as stat,
            tc.tile_pool(name="ps", bufs=3, space="PSUM") as ps,
            tc.tile_pool(name="psa", bufs=1, space="PSUM") as psap,
            tc.tile_pool(name="psw", bufs=1, space="PSUM") as pswp,
        ):
            # ---- input DMAs up front; wt first (longest dep chain) ----
            sm = persist.tile([3, NPAIR * L + 256], F16, tag="sm")
            nc.sync.dma_start(out=sm, in_=sm_in[:, :])
            r2 = sm[:, 0 : NPAIR * L].rearrange("k (s l) -> k s l", s=NPAIR)
            # l2 col = gb*128 + sp*64 + c  (contiguous per gb block)
            l2 = sm[:, NPAIR * L : NPAIR * L + 256]
            wt = persist.tile([64, 8192], F16, tag="wt")
            nc.sync.dma_start(out=wt[:, 0:4096], in_=wt_in[:, 0:4096])
            nc.sync.dma_start(out=wt[:, 4096:8192], in_=wt_in[:, 4096:8192])
            uit = persist.tile([64, BPC], F16, tag="uit")
            nc.sync.dma_start(out=uit, in_=ui_in[:, :])
            xet = []
            for s in range(NPAIR):
                t_ = xe.tile([128, 2 * L], F16, tag="xe", name=f"xe{s}")
                nc.sync.dma_start(out=t_, in_=xe_in[s, :, :])
                xet.append(t_)

            # block-diag lhsT for all pairs; free col = s*256 + gb*128 + sp*64 + c:
            #   lt[spk*64+q, s, gb, spm, c] = A[2s+spm, gb*64+c, q] iff spk==spm
            lt_all = persist.tile([128, NPAIR, 2, 2, CPC], F16, tag="lt_all")
            nc.gpsimd.memset(lt_all[:, :, :, :, :], 0.0)
            eps_t = persist.tile([128, 1], FP32, tag="eps")
            nc.vector.memset(eps_t, EPS)

            # PE warm-up while wt streams in (p-state ramp)
            for wu in range(4):
                pw = pswp.tile([BPC, 512], FP32, tag="psw", name=f"pw{wu}")
                nc.tensor.matmul(
                    pw, lhsT=sm[:, 0:BPC], rhs=sm[:, 0:512], start=True, stop=True
                )

            # ---- stage 1 (flipped): psa[c2, q*8+b] = A[b, q, c2] ----
            # 64 matmuls, M=128 (c2 on partitions), N=8, one psum bank.
            psa = psap.tile([128, 512], FP32, tag="psa", name="psa")
            for q in range(64):
                nc.tensor.matmul(
                    psa[:, q * 8 : q * 8 + 8],
                    lhsT=wt[:, q * 128 : (q + 1) * 128],
                    rhs=uit,
                    start=True,
                    stop=True,
                    skip_group_check=True,
                )
            # evac + col reorder (q,b) -> (b,q):  at7[c2, b*64+q] = A[b,q,c2]
            at7 = persist.tile([128, 512], F16, tag="at7")
            at7v = at7.rearrange("p (b q) -> p q b", b=BPC)
            psav = psa.rearrange("p (q b) -> p q b", q=64)
            nc.scalar.activation(out=at7v, in_=psav, func=AF.Copy)

            # ---- A redistribute: XBAR transpose per pair + 2 col-shift
            # copies into the zeroed block-diagonal tile ----
            for k in range(NPAIR):
                ltp = persist.tile([128, 128], F16, tag=f"ltp{k}")
                nc.scalar.dma_start_transpose(
                    out=ltp, in_=at7[:, k * 128 : (k + 1) * 128]
                )
                ltpv = ltp.rearrange("p (gb c) -> p gb c", gb=2)
                for sp in range(2):
                    rows = slice(sp * 64, sp * 64 + 64)
                    if sp == 0:
                        nc.vector.tensor_copy(
                            out=lt_all[rows, k, :, sp, :], in_=ltpv[rows, :, :]
                        )
                    else:
                        nc.gpsimd.tensor_copy(
                            out=lt_all[rows, k, :, sp, :], in_=ltpv[rows, :, :]
                        )

            # PE fillers bridge the redistribute gap (p-state ramp):
            # batch A becomes ready with wt's second half, batch B with at7,
            # so they run DURING the gap instead of jumping ahead of stage 1.
            for fl in range(6):
                pf = pswp.tile([BPC, 512], FP32, tag="psw", name=f"pfa{fl}")
                nc.tensor.matmul(
                    pf, lhsT=wt[:, 4096:4104], rhs=wt[:, 4096:4608],
                    start=True, stop=True,
                )
            for fl in range(10):
                pf = pswp.tile([BPC, 512], FP32, tag="psw", name=f"pfb{fl}")
                nc.tensor.matmul(
                    pf, lhsT=at7[:, 0:BPC], rhs=at7[:, 0:512],
                    start=True, stop=True,
                )

            # ---- norm stats for all pairs first (keeps in-order engine
            # queues from blocking later pairs' stats behind PSUM ops) ----
            mvs, rstds, xms = [], [], []
            for s in range(NPAIR):
                xt = xet[s][:, 0:L]
                st = stat.tile([128, 2, 6], FP32, tag="st", name=f"st{s}")
                nc.vector.bn_stats(st[:, 0, :], xt[:, 0:512])
                nc.vector.bn_stats(st[:, 1, :], xt[:, 512:1024])
                mv = stat.tile([128, 2], FP32, tag="mv", name=f"mv{s}")
                nc.vector.bn_aggr(mv, st)
                rstd = stat.tile([128, 1], FP32, tag="rstd", name=f"rstd{s}")
                nc.scalar.activation(
                    out=rstd, in_=mv[:, 1:2], func=AF.Sqrt, bias=eps_t, scale=1.0
                )
                nc.vector.reciprocal(rstd, rstd)
                xm = work.tile([128, L], F16, tag="xm", name=f"xm{s}")
                nc.vector.tensor_scalar(
                    out=xm, in0=xt, scalar1=mv[:, 0:1], scalar2=None,
                    op0=OP.subtract,
                )
                mvs.append(mv)
                rstds.append(rstd)
                xms.append(xm)

            # ---- stage 2 + combine, per sample pair ----
            od = out_d.rearrange("b c (h l) -> b c h l", h=2)
            for s in range(NPAIR):
                xt = xet[s][:, 0:L]
                et = xet[s][:, L : 2 * L]
                mv, rstd, xm = mvs[s], rstds[s], xms[s]

                pm = ps.tile([128, L], FP32, tag="ps", name=f"pm{s}")
                pq = ps.tile([128, L], FP32, tag="ps", name=f"pq{s}")
                for n in range(2):
                    cols = slice(n * 512, (n + 1) * 512)
                    nc.tensor.matmul(
                        pm[:, cols],
                        lhsT=lt_all[:, s, 0, :, :],
                        rhs=et[:, cols],
                        start=True,
                        stop=False,
                    )
                    nc.tensor.matmul(
                        pm[:, cols],
                        lhsT=l2[:, 0:128],
                        rhs=r2[:, s, cols],
                        start=False,
                        stop=True,
                    )
                    nc.tensor.matmul(
                        pq[:, cols],
                        lhsT=lt_all[:, s, 1, :, :],
                        rhs=et[:, cols],
                        start=True,
                        stop=False,
                    )
                    nc.tensor.matmul(
                        pq[:, cols],
                        lhsT=l2[:, 128:256],
                        rhs=r2[:, s, cols],
                        start=False,
                        stop=True,
                    )

                t1 = work.tile([128, L], F16, tag="t1")
                ot = work.tile([128, L], F16, tag="ot")
                if s < 3:
                    # msb = P*rstd (ACT), t1 = xm*msb (Pool),
                    # qsb = Q (ACT), out = t1 + qsb (DVE fp16 2x)
                    msb = work.tile([128, L], F16, tag="msb")
                    qsb = work.tile([128, L], F16, tag="qsb")
                    nc.scalar.activation(
                        out=msb, in_=pm, func=AF.Copy, scale=rstd
                    )
                    nc.gpsimd.tensor_tensor(
                        out=t1, in0=xm, in1=msb, op=OP.mult
                    )
                    nc.scalar.activation(out=qsb, in_=pq, func=AF.Copy)
                    nc.vector.tensor_add(out=ot, in0=t1, in1=qsb)
                    nc.sync.dma_start(out=out_d[2 * s : 2 * s + 2, :, :], in_=ot)
                else:
                    # last pair per half straight through DVE: short tail
                    for n in range(2):
                        cols = slice(n * 512, (n + 1) * 512)
                        nc.vector.scalar_tensor_tensor(
                            out=t1[:, cols], in0=xm[:, cols], scalar=rstd,
                            in1=pm[:, cols], op0=OP.mult, op1=OP.mult,
                        )
                        nc.vector.tensor_add(
                            out=ot[:, cols], in0=t1[:, cols], in1=pq[:, cols]
                        )
                        nc.sync.dma_start(
                            out=od[2 * s : 2 * s + 2, :, n, :], in_=ot[:, cols]
                        )

    return nc


def _prep_core_inputs(core, x, u_i, e_qid, t, W, V, bias):
    bg, cg = divmod(core, CG)
    bs = slice(bg * BPC, (bg + 1) * BPC)
    rg = slice(cg * CPC, (cg + 1) * CPC)
    rb = slice(C + cg * CPC, C + (cg + 1) * CPC)

    # xe: (NPAIR, 128, 2048) = [x pair rows | e pair rows]
    xp = x[bs, rg, :].reshape(NPAIR, 128, L)
    ep = e_qid[bs].reshape(NPAIR, 128, L)
    xe = np.concatenate([xp, ep], axis=2)

    w2 = np.concatenate([W[rg], W[rb]], axis=0)          # (128, 4096) c2=[g|b]
    # wtq[u, q*128 + c2] = w2[c2, u*64 + q]
    wtq = np.ascontiguousarray(
        w2.reshape(128, U, Q).transpose(1, 2, 0)
    ).reshape(64, 8192)

    uiT = np.ascontiguousarray(u_i[bs].T)                # (64, 8)

    vg, vb = V[rg, 0], V[rb, 0]
    bgm, bbt = bias[rg], bias[rb]
    # l2 col = gb*128 + sp*64 + c  (gamma block 0:128, beta block 128:256)
    l2 = np.zeros((3, 256), np.float32)
    l2[0, 0:64] = vg
    l2[1, 64:128] = vg
    l2[2, 0:64] = 1.0 + bgm
    l2[2, 64:128] = 1.0 + bgm
    l2[0, 128:192] = vb
    l2[1, 192:256] = vb
    l2[2, 128:192] = bbt
    l2[2, 192:256] = bbt

    r2 = np.empty((3, NPAIR, L), np.float32)
    for s in range(NPAIR):
        r2[0, s] = t[bg * BPC + 2 * s, 0]
        r2[1, s] = t[bg * BPC + 2 * s + 1, 0]
    r2[2] = 1.0
    sm = np.concatenate([r2.reshape(3, NPAIR * L), l2], axis=1)

    return {
        "xe_s": np.ascontiguousarray(xe).astype(np.float16),
        "wtq": wtq.astype(np.float16),
        "uiT": uiT.astype(np.float16),
        "sm2": np.ascontiguousarray(sm).astype(np.float16),
    }


def kernel(x, u_i, e_qid, t, W, V, bias):
    x = np.asarray(x, np.float32)
    u_i = np.asarray(u_i, np.float32)
    e_qid = np.asarray(e_qid, np.float32)
    t = np.asarray(t, np.float32)
    W = np.asarray(W, np.float32)
    V = np.asarray(V, np.float32)
    bias = np.asarray(bias, np.float32)

    if "nc" not in _CACHE:
        _CACHE["nc"] = _build_nc()
    nc = _CACHE["nc"]

    in_maps = [
        _prep_core_inputs(i, x, u_i, e_qid, t, W, V, bias) for i in range(N_CORES)
    ]
    results = run_bass_kernel_spmd(nc, in_maps, list(range(N_CORES))).results

    out = np.empty((B, C, L), np.float32)
    for i in range(N_CORES):
        bg, cg = divmod(i, CG)
        out[bg * BPC : (bg + 1) * BPC, cg * CPC : (cg + 1) * CPC, :] = results[i][
            "out_s"
        ].astype(np.float32)
    return out


# revision 15
# speedup vs baseline: 1.4588x; 1.4588x over previous
"""ConditionAwareAdaIN Trainium2 kernel (v5: flipped stage-1 + XBAR
transpose, fp16 I/O, engine-balanced elementwise).

Reference computation (B=16, C=256, L=1024, U=64, Q=64):
    nx    = InstanceNorm1d(x)                       # per-(b,c) stats over L
    A     = einsum('bu,cuq->bcq', u_i, W.reshape(2C,U,Q))
    style = einsum('bcq,bql->bcl', A, e_qid)
    gamma, beta = split(style + V@t + bias, 2, axis=1)
    out   = (1 + gamma) * nx + beta

Sharding: 2-way over batch x 4-way over channels -> 8 cores, each owning
8 samples x 64 channels (its slice of gamma AND beta rows of W/V/bias).

Key points:
  - all device I/O tensors are fp16 -> DMA bytes halve vs fp32; PSUM and
    stats stay fp32.  End-to-end rel err ~1e-3 vs the 2e-2 gate.
  - stage 1 is flipped: 64 matmuls out[c2(128), b(8)] (lhsT = W_q slice,
    rhs = u_i^T), all into ONE psum bank -> a single [128, 512]
    evacuation instead of 8x1024 columns of [8, *] copies.
  - the A redistribute is a DMA-engine XBAR transpose (SBUF->SBUF,
    one per sample pair) -- no DRAM bounce -- followed by two cheap
    partition-preserving copies per pair into the zeroed block-diagonal
    lhsT tile.
  - per-pair elementwise: ACT evacuates P (rstd folded) and Q to fp16,
    Pool does (x-mean)*msb, DVE does stats + the final fp16 2x add; the
    last pair runs per-half straight through DVE to shorten the tail.
"""

import json

import numpy as np

for _p in ("/opt/trn_rl_repo", "/root/.axon_site/_ro/trn_rl_repo"):
    import sys as _sys

    if _p not in _sys.path:
        _sys.path.append(_p)

import concourse.bass as bass
import concourse.mybir as mybir
from concourse.tile import TileContext
from concourse.bass_utils import run_bass_kernel_spmd


def _split_sync_waits(raw: bytes, keep: int = 1) -> bytes:
    """Walrus in this env accepts at most one sync wait per TPB instruction.

    Tile packs several waits into sync_info.on_wait; re-emit the excess as
    standalone single-wait EventSemaphore instructions (what wait_ge emits)
    immediately before the instruction, in the same engine stream.
    """
    bir = json.loads(raw)
    n = 0
    for fn in bir["functions"]:
        for blk in fn["blocks"]:
            out = []
            for ins in blk["instructions"]:
                si = ins.get("sync_info")
                ws = si.get("on_wait") if si else None
                if ws and len(ws) > keep:
                    for w in ws[: len(ws) - keep]:
                        n += 1
                        out.append(
                            {
                                "debug": ins.get("debug", 0),
                                "engine": ins["engine"],
                                "ins": [],
                                "outs": [],
                                "name": f"evw-{n}",
                                "opcode": "EventSemaphore",
                                "sync_info": {"on_update": [], "on_wait": [w]},
                            }
                        )
                    si["on_wait"] = ws[len(ws) - keep :]
                out.append(ins)
            blk["instructions"] = out
    return json.dumps(bir).encode()


class _Bass(bass.Bass):
    def to_json_bytes(self) -> bytes:
        return _split_sync_waits(super().to_json_bytes())


B, C, L = 16, 256, 1024
U, Q = 64, 64
EPS = 1e-5
N_CORES = 8
BG, CG = 2, 4          # batch groups x channel groups
BPC = B // BG          # samples per core = 8
CPC = C // CG          # channels per core = 64
NPAIR = BPC // 2       # sample pairs per core = 4

FP32 = mybir.dt.float32
F16 = mybir.dt.float16

_CACHE = {}


def _build_nc(detect_races: bool = True):
    nc = _Bass(detect_race_conditions=detect_races)

    # xe: per pair, rows = (2 samples x 64 ch/q), cols = [x (1024) | e (1024)]
    xe_in = nc.dram_tensor("xe_s", [NPAIR, 128, 2 * L], F16, kind="ExternalInput")
    # wtu: rows u(64), cols [uiT (8) | q*128 + c2 -> W2[c2, u, q] (8192)]
    wt_in = nc.dram_tensor("wtu", [64, BPC + 8192], F16, kind="ExternalInput")
    # identity for PE transposes
    id_in = nc.dram_tensor("idw", [128, 128], F16, kind="ExternalInput")
    # sm: [r2 (4 pairs x 1024) | l2 (256)] on 3 partitions
    sm_in = nc.dram_tensor("sm2", [3, NPAIR * L + 256], F16, kind="ExternalInput")
    out_d = nc.dram_tensor("out_s", [BPC, CPC, L], F16, kind="ExternalOutput")

    AF = mybir.ActivationFunctionType
    OP = mybir.AluOpType

    with TileContext(nc) as tc:
        with (
            tc.tile_pool(name="persist", bufs=1) as persist,
            tc.tile_pool(name="xe", bufs=4) as xe,
            tc.tile_pool(name="work", bufs=6) as work,
            tc.tile_pool(name="stat", bufs=8) as stat,
            tc.tile_pool(name="ps", bufs=3, space="PSUM") as ps,
            tc.tile_pool(name="psa", bufs=1, space="PSUM") as psap,
            tc.tile_pool(name="psw", bufs=1, space="PSUM") as pswp,
        ):
            # ---- input DMAs up front; wt first (longest dep chain) ----
            sm = persist.tile([3, NPAIR * L + 256], F16, tag="sm")
            nc.sync.dma_start(out=sm, in_=sm_in[:, :])
            r2 = sm[:, 0 : NPAIR * L].rearrange("k (s l) -> k s l", s=NPAIR)
            # l2 col = gb*128 + sp*64 + c  (contiguous per gb block)
            l2 = sm[:, NPAIR * L : NPAIR * L + 256]
            idt = persist.tile([128, 128], F16, tag="idt")
            nc.sync.dma_start(out=idt, in_=id_in[:, :])
            wtu = persist.tile([64, BPC + 8192], F16, tag="wtu")
            nc.sync.dma_start(out=wtu[:, 0 : BPC + 4096], in_=wt_in[:, 0 : BPC + 4096])
            nc.sync.dma_start(out=wtu[:, BPC + 4096 :], in_=wt_in[:, BPC + 4096 :])
            uit = wtu[:, 0:BPC]
            wt = wtu[:, BPC : BPC + 8192]
            xet = []
            for s in range(NPAIR):
                t_ = xe.tile([128, 2 * L], F16, tag="xe", name=f"xe{s}")
                nc.sync.dma_start(out=t_, in_=xe_in[s, :, :])
                xet.append(t_)

            # block-diag lhsT for all pairs; free col = s*256 + gb*128 + sp*64 + c:
            #   lt[spk*64+q, s, gb, spm, c] = A[2s+spm, gb*64+c, q] iff spk==spm
            lt_all = persist.tile([128, NPAIR, 2, 2, CPC], F16, tag="lt_all")
            nc.gpsimd.memset(lt_all[:, :, :, :, :], 0.0)
            eps_t = persist.tile([128, 1], FP32, tag="eps")
            nc.vector.memset(eps_t, EPS)

            # PE warm-up while wt streams in (p-state ramp)
            for wu in range(4):
                pw = pswp.tile([BPC, 512], FP32, tag="psw", name=f"pw{wu}")
                nc.tensor.matmul(
                    pw, lhsT=sm[:, 0:BPC], rhs=sm[:, 0:512], start=True, stop=True
                )

            # ---- stage 1 (flipped): psa[c2, q*8+b] = A[b, q, c2] ----
            # 64 matmuls, M=128 (c2 on partitions), N=8, one psum bank.
            psa = psap.tile([128, 512], FP32, tag="psa", name="psa")
            for q in range(64):
                nc.tensor.matmul(
                    psa[:, q * 8 : q * 8 + 8],
                    lhsT=wt[:, q * 128 : (q + 1) * 128],
                    rhs=uit,
                    start=True,
                    stop=True,
                    skip_group_check=True,
                )
            # evac + col reorder (q,b) -> (b,q):  at7[c2, b*64+q] = A[b,q,c2]
            at7 = persist.tile([128, 512], F16, tag="at7")
            at7v = at7.rearrange("p (b q) -> p q b", b=BPC)
            psav = psa.rearrange("p (q b) -> p q b", q=64)
            nc.scalar.activation(out=at7v, in_=psav, func=AF.Copy)

            # ---- A redistribute: PE transpose per pair (through PSUM),
            # one evac, then 2 col-shift copies per pair into the zeroed
            # block-diagonal tile.  No DMA involved.
            psl = psap.tile([128, 512], F16, tag="psa", name="psl")
            for k in range(NPAIR):
                nc.tensor.matmul(
                    psl[:, k * 128 : (k + 1) * 128],
                    lhsT=at7[:, k * 128 : (k + 1) * 128],
                    rhs=idt,
                    is_transpose=True,
                    skip_group_check=True,
                )
            ltp = persist.tile([128, 512], F16, tag="ltp")
            nc.scalar.activation(out=ltp, in_=psl, func=AF.Copy)
            ltpv = ltp.rearrange("p (k gb c) -> p k gb c", k=NPAIR, gb=2)
            for k in range(NPAIR):
                for sp in range(2):
                    rows = slice(sp * 64, sp * 64 + 64)
                    if sp == 0:
                        nc.vector.tensor_copy(
                            out=lt_all[rows, k, :, sp, :], in_=ltpv[rows, k, :, :]
                        )
                    else:
                        nc.gpsimd.tensor_copy(
                            out=lt_all[rows, k, :, sp, :], in_=ltpv[rows, k, :, :]
                        )

            # PE fillers bridge the redistribute gap (p-state ramp):
            # ready with wt's second half / at7, so they run in the gap.
            for fl in range(6):
                pf = pswp.tile([BPC, 512], FP32, tag="psw", name=f"pfa{fl}")
                nc.tensor.matmul(
                    pf, lhsT=wt[:, 4096:4104], rhs=wt[:, 4096:4608],
                    start=True, stop=True,
                )
            for fl in range(8):
                pf = pswp.tile([BPC, 512], FP32, tag="psw", name=f"pfb{fl}")
                nc.tensor.matmul(
                    pf, lhsT=at7[:, 0:BPC], rhs=at7[:, 0:512],
                    start=True, stop=True,
                )

            # ---- norm stats for all pairs first (keeps in-order engine
            # queues from blocking later pairs' stats behind PSUM ops) ----
            mvs, rstds, xms = [], [], []
            for s in range(NPAIR):
                xt = xet[s][:, 0:L]
                st = stat.tile([128, 2, 6], FP32, tag="st", name=f"st{s}")
                nc.vector.bn_stats(st[:, 0, :], xt[:, 0:512])
                nc.vector.bn_stats(st[:, 1, :], xt[:, 512:1024])
                mv = stat.tile([128, 2], FP32, tag="mv", name=f"mv{s}")
                nc.vector.bn_aggr(mv, st)
                rstd = stat.tile([128, 1], FP32, tag="rstd", name=f"rstd{s}")
                nc.scalar.activation(
                    out=rstd, in_=mv[:, 1:2], func=AF.Sqrt, bias=eps_t, scale=1.0
                )
                nc.vector.reciprocal(rstd, rstd)
                xm = work.tile([128, L], F16, tag="xm", name=f"xm{s}")
                nc.vector.tensor_scalar(
                    out=xm, in0=xt, scalar1=mv[:, 0:1], scalar2=None,
                    op0=OP.subtract,
                )
                mvs.append(mv)
                rstds.append(rstd)
                xms.append(xm)

            # ---- stage 2 + combine, per sample pair ----
            od = out_d.rearrange("b c (h l) -> b c h l", h=2)
            for s in range(NPAIR):
                xt = xet[s][:, 0:L]
                et = xet[s][:, L : 2 * L]
                mv, rstd, xm = mvs[s], rstds[s], xms[s]

                pm = ps.tile([128, L], FP32, tag="ps", name=f"pm{s}")
                pq = ps.tile([128, L], FP32, tag="ps", name=f"pq{s}")
                for n in range(2):
                    cols = slice(n * 512, (n + 1) * 512)
                    nc.tensor.matmul(
                        pm[:, cols],
                        lhsT=lt_all[:, s, 0, :, :],
                        rhs=et[:, cols],
                        start=True,
                        stop=False,
                    )
                    nc.tensor.matmul(
                        pm[:, cols],
                        lhsT=l2[:, 0:128],
                        rhs=r2[:, s, cols],
                        start=False,
                        stop=True,
                    )
                    nc.tensor.matmul(
                        pq[:, cols],
                        lhsT=lt_all[:, s, 1, :, :],
                        rhs=et[:, cols],
                        start=True,
                        stop=False,
                    )
                    nc.tensor.matmul(
                        pq[:, cols],
                        lhsT=l2[:, 128:256],
                        rhs=r2[:, s, cols],
                        start=False,
                        stop=True,
                    )

                t1 = work.tile([128, L], F16, tag="t1")
                ot = work.tile([128, L], F16, tag="ot")
                if s < 3:
                    # msb = P*rstd (ACT), t1 = xm*msb (Pool),
                    # qsb = Q (ACT), out = t1 + qsb (DVE fp16 2x)
                    msb = work.tile([128, L], F16, tag="msb")
                    qsb = work.tile([128, L], F16, tag="qsb")
                    nc.scalar.activation(
                        out=msb, in_=pm, func=AF.Copy, scale=rstd
                    )
                    nc.gpsimd.tensor_tensor(
                        out=t1, in0=xm, in1=msb, op=OP.mult
                    )
                    nc.scalar.activation(out=qsb, in_=pq, func=AF.Copy)
                    nc.vector.tensor_add(out=ot, in0=t1, in1=qsb)
                    nc.sync.dma_start(out=out_d[2 * s : 2 * s + 2, :, :], in_=ot)
                else:
                    # last pair per half straight through DVE: short tail
                    for n in range(2):
                        cols = slice(n * 512, (n + 1) * 512)
                        nc.vector.scalar_tensor_tensor(
                            out=t1[:, cols], in0=xm[:, cols], scalar=rstd,
                            in1=pm[:, cols], op0=OP.mult, op1=OP.mult,
                        )
                        nc.vector.tensor_add(
                            out=ot[:, cols], in0=t1[:, cols], in1=pq[:, cols]
                        )
                        nc.sync.dma_start(
                            out=od[2 * s : 2 * s + 2, :, n, :], in_=ot[:, cols]
                        )

    return nc


def _prep_core_inputs(core, x, u_i, e_qid, t, W, V, bias):
    bg, cg = divmod(core, CG)
    bs = slice(bg * BPC, (bg + 1) * BPC)
    rg = slice(cg * CPC, (cg + 1) * CPC)
    rb = slice(C + cg * CPC, C + (cg + 1) * CPC)

    # xe: (NPAIR, 128, 2048) = [x pair rows | e pair rows]
    xp = x[bs, rg, :].reshape(NPAIR, 128, L)
    ep = e_qid[bs].reshape(NPAIR, 128, L)
    xe = np.concatenate([xp, ep], axis=2)

    w2 = np.concatenate([W[rg], W[rb]], axis=0)          # (128, 4096) c2=[g|b]
    # wtu = [uiT | wtq] with wtq[u, q*128 + c2] = w2[c2, u*64 + q]
    wtq = np.ascontiguousarray(
        w2.reshape(128, U, Q).transpose(1, 2, 0)
    ).reshape(64, 8192)
    wtu = np.concatenate([u_i[bs].T, wtq], axis=1)       # (64, 8+8192)

    vg, vb = V[rg, 0], V[rb, 0]
    bgm, bbt = bias[rg], bias[rb]
    # l2 col = gb*128 + sp*64 + c  (gamma block 0:128, beta block 128:256)
    l2 = np.zeros((3, 256), np.float32)
    l2[0, 0:64] = vg
    l2[1, 64:128] = vg
    l2[2, 0:64] = 1.0 + bgm
    l2[2, 64:128] = 1.0 + bgm
    l2[0, 128:192] = vb
    l2[1, 192:256] = vb
    l2[2, 128:192] = bbt
    l2[2, 192:256] = bbt

    r2 = np.empty((3, NPAIR, L), np.float32)
    for s in range(NPAIR):
        r2[0, s] = t[bg * BPC + 2 * s, 0]
        r2[1, s] = t[bg * BPC + 2 * s + 1, 0]
    r2[2] = 1.0
    sm = np.concatenate([r2.reshape(3, NPAIR * L), l2], axis=1)

    return {
        "xe_s": np.ascontiguousarray(xe).astype(np.float16),
        "wtu": np.ascontiguousarray(wtu).astype(np.float16),
        "idw": np.eye(128, dtype=np.float16),
        "sm2": np.ascontiguousarray(sm).astype(np.float16),
    }


def kernel(x, u_i, e_qid, t, W, V, bias):
    x = np.asarray(x, np.float32)
    u_i = np.asarray(u_i, np.float32)
    e_qid = np.asarray(e_qid, np.float32)
    t = np.asarray(t, np.float32)
    W = np.asarray(W, np.float32)
    V = np.asarray(V, np.float32)
    bias = np.asarray(bias, np.float32)

    if "nc" not in _CACHE:
        _CACHE["nc"] = _build_nc()
    nc = _CACHE["nc"]

    in_maps = [
        _prep_core_inputs(i, x, u_i, e_qid, t, W, V, bias) for i in range(N_CORES)
    ]
    results = run_bass_kernel_spmd(nc, in_maps, list(range(N_CORES))).results

    out = np.empty((B, C, L), np.float32)
    for i in range(N_CORES):
        bg, cg = divmod(i, CG)
        out[bg * BPC : (bg + 1) * BPC, cg * CPC : (cg + 1) * CPC, :] = results[i][
            "out_s"
        ].astype(np.float32)
    return out
